# revision 1
# baseline (speedup 1.0000x reference)
# GCN layer kernel for Trainium2: out[b] = relu((a[b] @ x[b]) @ W) * mask[b]
#
# Sharding: data-parallel over the batch (graph) dim. B=8 graphs, 8 cores,
# one graph per core; W replicated. Inputs are the FULL tensors; shards are
# sliced host-side and the per-core outputs stacked back together.
#
# Per-core dataflow (a: [2048,2048], x: [2048,512], W: [512,512]):
#   - All matmuls run in bf16: 1 cycle/row on the PE at full clock (same
#     rate as f32r) and 1 cycle/row for PE transposes (fp32 pays 2).
#     rel-err ~3e-3 vs the fp32 reference; tolerance is 2e-2.
#   - a loads as fp32 [128,2048] strips (2 per chunk on each HWDGE
#     queue, chunks in order c0, c1 up front, c2/c3 prefetched from the
#     chunk bodies), DVE casts them to bf16.
#   - The PE transposes a through PSUM in quads of [128,128] bf16 tiles
#     (4 per bank; DVE/ACT copy back into at[p=m%128, mtile, ni, r]).
#     All 256 tile transposes cost ~13.6us of PE time; the DMA fabric
#     (~360-430 GB/s/core shared, measured) carries only real traffic.
#       mm1: tT[f,n] = sum_m x[m,f] * aT[m,n]   (lhsT = x, rhs = aT)
#       mm2: out[n,d] = sum_f tT[f,n] * W[f,d]  (lhsT = tT, rhs = W)
#   - HAM clock gate (measured): PE activity sampled in 3.41us epochs; an
#     epoch with a multi-us idle gap drops the clock to k=4/8 (matmuls
#     379ns instead of 213ns). The schedule keeps the PE stream gapless:
#     warm-ups cover the initial DMA window, chunk j+1's transposes ride
#     the back half of chunk j's mm1 stream, and mm2 runs a full chunk
#     behind mm1. The PE queue is in-order: nothing that waits on late
#     data may be emitted ahead of work whose inputs are already there.
#   - x loads as 16 contiguous row-tiles [128,512] (row-tile == mi):
#     row-major loads cost ~128 DMA descriptors each, unlike a
#     column-strided rearrange of x which runs as a multi-us inline
#     transfer on the issuing engine and stalls the a-strips behind it.
#     mask[n] = any(x[n,:] != 0) via |x| row-sums (ACT), applied as the
#     scale input of the fused ReLU.
#
# Queues: sync/scalar = a strips (2+2 per chunk) + x row-tiles; output
#   stores ride sync late (its loads are done by then; the gpsimd
#   software-DGE ring adds ~4us of end-of-run drain if used for stores);
#   gpsimd only carries w (cast-DMA to bf16 in flight). PSUM: 4 mm1 + 2
#   transpose + 2 mm2 banks (mm2 in two ns-pair waves; warms borrow the
#   mm2 pool, the final wave borrows idle mm1 banks to avoid a WAR stall).

import numpy as np

B, N, F, D = 8, 2048, 512, 512
P = 128
NT = N // P        # 16 row-tiles of n (and of m, since a is square)
FT = F // P        # 4 tiles of f
NCHUNK = 512       # n is processed in chunks of 512 rows
NJ = N // NCHUNK   # 4
NSUB = NCHUNK // P # 4

_CACHE = {}


def _build_nc():
    from contextlib import ExitStack

    from concourse import bacc, mybir, tile
    from concourse.masks import make_identity

    f32 = mybir.dt.float32
    bf16 = mybir.dt.bfloat16
    AF = mybir.ActivationFunctionType

    nc = bacc.Bacc(None)
    a_d = nc.dram_tensor("a", [N, N], f32, kind="ExternalInput")
    x_d = nc.dram_tensor("x", [N, F], f32, kind="ExternalInput")
    w_d = nc.dram_tensor("kernel", [F, D], f32, kind="ExternalInput")
    o_d = nc.dram_tensor("out", [N, D], f32, kind="ExternalOutput")

    with tile.TileContext(nc) as tc, ExitStack() as ctx:
        const = ctx.enter_context(tc.tile_pool(name="const", bufs=1))
        xp = ctx.enter_context(tc.tile_pool(name="xp", bufs=1))
        wp = ctx.enter_context(tc.tile_pool(name="wp", bufs=1))
        xs = ctx.enter_context(tc.tile_pool(name="xs", bufs=16))
        afp = ctx.enter_context(tc.tile_pool(name="afp", bufs=4))
        abp = ctx.enter_context(tc.tile_pool(name="abp", bufs=12))
        atp = ctx.enter_context(tc.tile_pool(name="atp", bufs=2))
        ttp = ctx.enter_context(tc.tile_pool(name="ttp", bufs=2))
        outp = ctx.enter_context(tc.tile_pool(name="outp", bufs=4))
        scr = ctx.enter_context(tc.tile_pool(name="scr", bufs=2))
        ps_mm = ctx.enter_context(tc.tile_pool(name="ps_mm", bufs=4, space="PSUM"))
        ps_tp = ctx.enter_context(tc.tile_pool(name="ps_tp", bufs=2, space="PSUM"))
        ps_o = ctx.enter_context(tc.tile_pool(name="ps_o", bufs=2, space="PSUM"))

        ident = const.tile([P, P], f32)
        make_identity(nc, ident[:])
        ident_b = const.tile([P, P], bf16)
        nc.vector.tensor_copy(ident_b[:], ident[:])

        def warm_fp32():
            # fp32 identity matmul: counts as HAM activity, output unused.
            pw = ps_o.tile([P, D], f32, tag="pso", name="pw")
            nc.tensor.matmul(
                pw[:, :P], lhsT=ident[:], rhs=ident[:], start=True, stop=True
            )

        def warm_bf16(lhs, rhs):
            # bf16 warm matmul on freshly-cast tiles: fires as the cast
            # lands, pacing PE activity through the DMA wait.
            pw = ps_o.tile([P, D], f32, tag="pso", name="pwb")
            nc.tensor.matmul(
                pw[:, : rhs.shape[-1]], lhsT=lhs, rhs=rhs, start=True, stop=True
            )

        for _ in range(14):
            warm_fp32()

        x_b = xp.tile([P, NT, F], bf16)
        w_b = wp.tile([P, FT, D], bf16)
        sumabs = const.tile([P, NT], f32)
        mask_sb = const.tile([P, NT], f32)

        ab_strips = [[None] * NSUB for _ in range(NJ)]
        at_tiles = [None] * NJ
        cbn = 0  # quad-copyback counter for DVE/ACT alternation

        def load_strip(nj, ni):
            queue = nc.sync if ni < 2 else nc.scalar
            af = afp.tile([P, N], f32, tag="af", name="af")
            r0 = (nj * NSUB + ni) * P
            queue.dma_start(af[:], a_d[r0 : r0 + P, :])
            return af

        def cast_strip(nj, ni, af, warm=False):
            ab = abp.tile([P, N], bf16, tag="ab", name="ab")
            nc.vector.tensor_copy(ab[:], af[:])
            ab_strips[nj][ni] = ab
            if warm:
                warm_bf16(ab[:, 0:P], ab[:, 0:NCHUNK])
                warm_bf16(ab[:, P : 2 * P], ab[:, NCHUNK : 2 * NCHUNK])

        def t_quad(nj, slot):
            # PE-transpose 4 tiles (strip ni, m-tiles q*4..q*4+3) through one
            # PSUM bank, then DVE/ACT copy into at[p, mtile, ni, r].
            nonlocal cbn
            ni, q = divmod(slot, 4)
            if at_tiles[nj] is None:
                at_tiles[nj] = atp.tile([P, NT, NSUB, P], bf16, tag="at", name="at")
            ps = ps_tp.tile([P, NCHUNK], bf16, tag="pst", name="pst")
            ab = ab_strips[nj][ni]
            for k in range(4):
                mi = q * 4 + k
                nc.tensor.transpose(
                    ps[:, k * P : (k + 1) * P], ab[:, mi * P : (mi + 1) * P],
                    ident_b[:],
                )
            # pinned to DVE: ACT's reductions/ReLUs must never delay the
            # aT tiles the next chunk's mm1 needs
            nc.vector.tensor_copy(
                at_tiles[nj][:, q * 4 : (q + 1) * 4, ni, :],
                ps[:].rearrange("p (a f) -> p a f", a=4),
            )

        # ---- preamble ----
        # HWDGE queue order = emission order; earliest-needed bytes first.
        af0 = [load_strip(0, ni) for ni in range(NSUB)]
        xls = []

        def load_xo(o):
            xl = xs.tile([P, F], f32, tag="xl", name=f"xl{o}")
            q = nc.sync if o % 2 == 0 else nc.scalar
            q.dma_start(xl[:], x_d[o * P : (o + 1) * P, :])
            xls.append(xl)

        # x right behind chunk 0: every fi-block of mm1 needs all 16 x
        # row-tiles as lhsT, so x gates chunk 0's matmuls; chunk 1 is only
        # needed by the transposes riding the back half of chunk 0's body.
        for o in range(NT):
            load_xo(o)
        af1 = [load_strip(1, ni) for ni in range(NSUB)]
        nc.gpsimd.dma_start(w_b[:], w_d[:].rearrange("(o p) d -> p o d", p=P))

        cast_strip(0, 0, af0[0], warm=True)
        cast_strip(0, 2, af0[2], warm=True)
        for slot in (0, 1, 8, 9):
            t_quad(0, slot)
        cast_strip(0, 1, af0[1], warm=True)
        for slot in (2, 3, 4, 5):
            t_quad(0, slot)
        cast_strip(0, 3, af0[3], warm=True)
        for slot in (10, 11, 6, 7):
            t_quad(0, slot)
        for o in range(4):
            nc.vector.tensor_copy(x_b[:, o, :], xls[o][:])
        for slot in (12, 13, 14, 15):
            t_quad(0, slot)
        warm_bf16(x_b[:, 0, 0:P], x_b[:, 0, :])
        cast_strip(1, 0, af1[0])
        cast_strip(1, 1, af1[1])
        cast_strip(1, 2, af1[2])
        cast_strip(1, 3, af1[3])
        for o in range(4, NT):
            nc.vector.tensor_copy(x_b[:, o, :], xls[o][:])
        for o in range(NT):
            abs_scr = scr.tile([P, F], bf16, tag="abs_scr")
            nc.scalar.activation(
                abs_scr[:], x_b[:, o, :], AF.Abs, accum_out=sumabs[:, o : o + 1]
            )
        nc.vector.tensor_scalar(
            mask_sb[:], sumabs[:], 0.0, None, mybir.AluOpType.is_gt
        )

        # ---- main loop ----
        tts = [None] * NJ
        po_banks = {}

        def mm2_wave(nj, w):
            # half of mm2 for chunk nj: output tiles ns = 2w, 2w+1
            # accumulated over all fi in 2 PSUM banks, then fused
            # relu(mask * po) -> SBUF -> store via the gpsimd queue.
            # The very last wave borrows idle mm1 banks: reusing ps_o
            # would WAR-stall on the previous wave's ReLU read.
            tt = tts[nj]
            pool, tg = (ps_mm, "psm") if (nj == NJ - 1 and w == 1) else (ps_o, "pso")
            po_banks[nj] = pos = [
                pool.tile([P, D], f32, tag=tg, name=f"po_{nj}_{w}_{i}")
                for i in range(2)
            ]
            for fi in range(FT):
                for i in range(2):
                    ns = 2 * w + i
                    nc.tensor.matmul(
                        pos[i][:],
                        lhsT=tt[:, fi, ns * P : (ns + 1) * P],
                        rhs=w_b[:, fi],
                        start=(fi == 0),
                        stop=(fi == FT - 1),
                    )
            for i in range(2):
                ns = 2 * w + i
                ni = nj * NSUB + ns
                ob = outp.tile([P, D], f32, tag="ob")
                nc.scalar.activation(
                    ob[:], pos[i][:], AF.Relu, scale=mask_sb[:, ni : ni + 1]
                )
                nc.sync.dma_start(o_d[ni * P : (ni + 1) * P, :], ob[:])

        def abs_batch(os):
            for o in os:
                abs_scr = scr.tile([P, F], bf16, tag="abs_scr")
                nc.scalar.activation(
                    abs_scr[:], x_b[:, o, :], AF.Abs,
                    accum_out=sumabs[:, o : o + 1],
                )

        # chunk 0: fi-outer; chunk 1's transposes ride fi>=2.
        af2 = [load_strip(2, ni) for ni in range(NSUB)]
        for ni in range(NSUB):
            cast_strip(2, ni, af2[ni])
        at_cur = at_tiles[0]
        tt0 = ttp.tile([P, FT, NCHUNK], bf16, tag="tt")
        tts[0] = tt0
        pt = [
            ps_mm.tile([P, NCHUNK], f32, tag="psm", name=f"pt_0_{fi}")
            for fi in range(FT)
        ]
        slots = iter(range(16))
        for fi in range(FT):
            for mi in range(NT):
                nc.tensor.matmul(
                    pt[fi][:],
                    lhsT=x_b[:, mi, fi * P : (fi + 1) * P],
                    rhs=at_cur[:, mi, :, :],
                    start=(mi == 0),
                    stop=(mi == NT - 1),
                )
                if fi >= 2 and mi % 2 == 1:
                    t_quad(1, next(slots))
            nc.scalar.copy(tt0[:, fi], pt[fi][:])

        def boundary_fill(n):
            # dummy matmuls on always-ready operands at the chunk
            # boundaries: if the next chunk's aT tiles are a couple of us
            # late, these keep the HAM epoch dense so the clock holds
            # k=8/8 (a dropped epoch costs ~7us of half-speed matmuls).
            # Each fill WAR-waits at most one just-emitted tt copyback.
            for _ in range(n):
                pd = ps_mm.tile([P, NCHUNK], f32, tag="psm", name="bfill")
                nc.tensor.matmul(
                    pd[:], lhsT=x_b[:, 0, 0:P], rhs=x_b[:, 0, :],
                    start=True, stop=True,
                )

        boundary_fill(10)

        # chunks 1..3: fi-outer; chunk nj+1's transposes ride fi>=2; mm2
        # of chunk nj-1 runs as two ns-pair waves at fi==1 / fi==3.
        for nj in range(1, NJ):
            if nj == 1:  # prefetch the last chunk
                af_n = [load_strip(3, ni) for ni in range(NSUB)]
                for ni in range(NSUB):
                    cast_strip(3, ni, af_n[ni])

            at_cur = at_tiles[nj]
            tt = ttp.tile([P, FT, NCHUNK], bf16, tag="tt")
            tts[nj] = tt
            pt = [
                ps_mm.tile([P, NCHUNK], f32, tag="psm", name=f"pt_{nj}_{fi}")
                for fi in range(FT)
            ]
            slots = iter(range(16))
            for fi in range(FT):
                for mi in range(NT):
                    nc.tensor.matmul(
                        pt[fi][:],
                        lhsT=x_b[:, mi, fi * P : (fi + 1) * P],
                        rhs=at_cur[:, mi, :, :],
                        start=(mi == 0),
                        stop=(mi == NT - 1),
                    )
                    if fi >= 2 and mi % 2 == 1 and nj + 1 < NJ:
                        t_quad(nj + 1, next(slots))
                nc.scalar.copy(tt[:, fi], pt[fi][:])
                if fi == 1:
                    mm2_wave(nj - 1, 0)
                elif fi == 3:
                    mm2_wave(nj - 1, 1)
            if nj in (1, 2):
                boundary_fill(4)

        mm2_wave(NJ - 1, 0)
        mm2_wave(NJ - 1, 1)

    nc.compile()
    return nc


def get_nc():
    if "nc" not in _CACHE:
        _CACHE["nc"] = _build_nc()
    return _CACHE["nc"]


def kernel(**inputs) -> np.ndarray:
    from concourse.bass_utils import run_bass_kernel_spmd

    x = np.ascontiguousarray(np.asarray(inputs["x"], dtype=np.float32))
    a = np.ascontiguousarray(np.asarray(inputs["a"], dtype=np.float32))
    w = np.ascontiguousarray(np.asarray(inputs["kernel"], dtype=np.float32))
    assert x.shape == (B, N, F) and a.shape == (B, N, N) and w.shape == (F, D)

    nc = get_nc()
    in_maps = [{"a": a[b], "x": x[b], "kernel": w} for b in range(B)]
    res = run_bass_kernel_spmd(nc, in_maps, core_ids=list(range(B)))
    return np.stack([res.results[b]["out"] for b in range(B)], axis=0)

